# revision 25
# baseline (speedup 1.0000x reference)
"""BinaryLinear (8192x4096 @ 4096x4096 binarized) on 8 TRN2 NeuronCores.

Strategy (tensor-parallel, column sharding per out_features):
  - Shard W/alpha/b along out_features: each core gets 512 output channels.
  - Replicate x (host pre-transposed to [in_f, n_rows] so the contraction
    dim lands on SBUF partitions without any device-side transpose).
  - Host gathers the 8 [8192, 512] shards with a concatenate on axis 1.

Default variant — fp8e4 DoubleRow (measured ~387us vs 503us bf16
baseline):
  - The binarized weights sign(W) are EXACT in fp8 (+-1); alpha and bias
    are applied in the f32 DVE epilogue, so weight precision is perfect.
  - x is shipped as a hi e4m3 plane over all of K plus a residual e4m3
    plane over the first 10/16 of K, both accumulated into the same PSUM
    group. fp8 products are exact on the PE (e6m3 upcast, e10m10 products)
    so the only error is e4m3 quantization of x on the UNcorrected 6/16 of
    K: rel err = 2.654e-2 * sqrt(6/16) = 1.626e-2 < 2e-2 gate, verified
    bit-exact against numpy on hardware.
  - DoubleRow perf mode packs 2 fp8 weights per PE cell: each matmul
    consumes a PAIR of k-tiles (stationary [128,2,128], moving [128,2,512])
    in 512 cycles -> 2x bf16 MAC throughput (216ns/MM measured). Total
    stream = (16 hi + 10 lo) pairs x 64 row-blocks = 1664 MMs ~ 359us.
  - Scheduling: whole n-chunk of x tiles resident in SBUF (per-kp tags,
    double buffered), ns-outer/k-inner loop so PSUM drains pipeline with
    compute; 8 PSUM banks via per-ns tags; weights on scalar+gpsimd DMA
    queues, x stream on sync queue, outputs on scalar queue.
  - drF additionally ships x/W pre-tiled on the host so each DMA
    partition line is one contiguous 1KB packet (the x queue is
    packet-rate limited at ~330 packets/us; natural layout was within 5%
    of that limit), warms the HAM clock gate with dummy matmuls on zeroed
    scratch during queue init, and drops the alpha multiply from the
    epilogue when alpha == 1 (checked on the host; general path kept).

Fallback variants kept for reference: drA (same math, natural layout),
hb/hb2 (host-binarized bf16, ~455-485us, rel err 1.7e-3),
dr1/dr15/dr2/drB/drC (other fp8 splits).
"""

import os
import sys

sys.path.insert(0, "/opt/trn_rl_repo")

import numpy as np

from concourse import bacc, bass, mybir
import concourse.tile as tile
from concourse.bass_utils import run_bass_kernel_spmd

N_ROWS = 8192
IN_F = 4096
OUT_F = 4096
N_CORES = 8
O_SHARD = OUT_F // N_CORES  # 512

P = 128

VARIANT = "drF"  # fp8 DoubleRow, 10/16 residual correction, pre-tiled DMA


def build_nc_hb(
    n_rows=N_ROWS,
    in_f=IN_F,
    o_shard=O_SHARD,
    n_chunk=512,
    x_bufs=8,
):
    """Per-core Bass graph, host-binarized bf16 weights (SPMD on all cores)."""
    f32 = mybir.dt.float32
    bf16 = mybir.dt.bfloat16

    assert in_f % P == 0 and n_rows % n_chunk == 0 and n_chunk % P == 0
    OCH = max(1, o_shard // 512)  # 512-wide o-chunks (one PSUM bank each)
    o_mm = o_shard // OCH
    assert o_mm <= 512 and o_mm * OCH == o_shard
    KO = in_f // P
    NCH = n_rows // n_chunk
    NS = n_chunk // P
    psum_bufs = 2 if NS * OCH * 2 <= 8 else 1
    assert NS * OCH * psum_bufs <= 8

    nc = bacc.Bacc("TRN2", target_bir_lowering=False)

    xT = nc.declare_dram_parameter("xT", [in_f, n_rows], bf16, isOutput=False)
    Wb = nc.declare_dram_parameter("Wb", [in_f, o_shard], bf16, isOutput=False)
    b_rep = nc.declare_dram_parameter("b_rep", [P, o_shard], f32, isOutput=False)
    out = nc.declare_dram_parameter("out", [n_rows, o_shard], f32, isOutput=True)

    xT_t = xT[:].rearrange("(ko p) n -> ko p n", p=P)
    Wb_t = Wb[:].rearrange("(ko p) o -> p ko o", p=P)

    with tile.TileContext(nc) as tc:
        with (
            tc.tile_pool(name="consts", bufs=1) as consts,
            tc.tile_pool(name="xp", bufs=x_bufs) as xp,
            tc.tile_pool(name="outp", bufs=4) as outp,
            tc.tile_pool(name="psum", bufs=psum_bufs, space="PSUM") as psump,
        ):
            # Weight/bias loads go through the scalar+gpsimd HWDGE queues so
            # the x-tile stream (sync queue) isn't stuck behind them.
            b_sb = consts.tile([P, o_shard], f32)
            nc.scalar.dma_start(out=b_sb[:], in_=b_rep[:])

            W_mm = consts.tile([P, KO, o_shard], bf16)
            for ko in range(KO):
                w_eng = nc.scalar if ko % 2 == 0 else nc.gpsimd
                w_eng.dma_start(out=W_mm[:, ko], in_=Wb_t[:, ko])

            for nch in range(NCH):
                psums = [
                    [
                        psump.tile(
                            [P, o_mm], f32,
                            tag=f"ps{ns}_{och}", name=f"ps{ns}_{och}",
                        )
                        for och in range(OCH)
                    ]
                    for ns in range(NS)
                ]
                for k in range(KO):
                    x_t = xp.tile([P, n_chunk], bf16, tag="xt")
                    nc.sync.dma_start(
                        out=x_t[:],
                        in_=xT_t[k, :, nch * n_chunk : (nch + 1) * n_chunk],
                    )
                    for ns in range(NS):
                        for och in range(OCH):
                            nc.tensor.matmul(
                                psums[ns][och][:],
                                x_t[:, ns * P : (ns + 1) * P],
                                W_mm[:, k, och * o_mm : (och + 1) * o_mm],
                                start=(k == 0),
                                stop=(k == KO - 1),
                            )
                for ns in range(NS):
                    o_sb = outp.tile([P, o_shard], f32, tag="o")
                    for och in range(OCH):
                        nc.vector.tensor_tensor(
                            o_sb[:, och * o_mm : (och + 1) * o_mm],
                            psums[ns][och][:],
                            b_sb[:, och * o_mm : (och + 1) * o_mm],
                            mybir.AluOpType.add,
                        )
                    row0 = nch * n_chunk + ns * P
                    nc.sync.dma_start(
                        out=out[row0 : row0 + P, :], in_=o_sb[:]
                    )
    nc.compile()
    return nc


def build_nc_hb2(
    n_rows=N_ROWS,
    in_f=IN_F,
    o_shard=O_SHARD,
    n_chunk=512,
):
    """Tuned host-binarized bf16 variant.

    vs build_nc_hb:
      - x tiles for a whole n-chunk stay resident in SBUF (per-k tags,
        double-buffered across chunks) and the matmul loop is ns-outer /
        k-inner, so each PSUM group finishes ~n_chunk/P times earlier and
        its DVE epilogue + output DMA overlap the next group's matmuls
        (shrinks the end-of-kernel drain tail).
      - output DMAs go on the scalar queue (idle after the weight load)
        instead of the sync queue, so they never delay the x-tile stream
        that feeds LDWEIGHTS at chunk boundaries.
    """
    f32 = mybir.dt.float32
    bf16 = mybir.dt.bfloat16

    assert in_f % P == 0 and n_rows % n_chunk == 0 and n_chunk % P == 0
    OCH = max(1, o_shard // 512)
    o_mm = o_shard // OCH
    assert o_mm <= 512 and o_mm * OCH == o_shard
    KO = in_f // P
    NCH = n_rows // n_chunk
    NS = n_chunk // P
    psum_bufs = 2 if NS * OCH * 2 <= 8 else 1
    assert NS * OCH * psum_bufs <= 8

    nc = bacc.Bacc("TRN2", target_bir_lowering=False)

    xT = nc.declare_dram_parameter("xT", [in_f, n_rows], bf16, isOutput=False)
    Wb = nc.declare_dram_parameter("Wb", [in_f, o_shard], bf16, isOutput=False)
    b_rep = nc.declare_dram_parameter("b_rep", [P, o_shard], f32, isOutput=False)
    out = nc.declare_dram_parameter("out", [n_rows, o_shard], f32, isOutput=True)

    xT_t = xT[:].rearrange("(ko p) n -> ko p n", p=P)
    Wb_t = Wb[:].rearrange("(ko p) o -> p ko o", p=P)

    with tile.TileContext(nc) as tc:
        with (
            tc.tile_pool(name="consts", bufs=1) as consts,
            tc.tile_pool(name="xp", bufs=2) as xp,
            tc.tile_pool(name="outp", bufs=4) as outp,
            tc.tile_pool(name="psum", bufs=psum_bufs, space="PSUM") as psump,
        ):
            b_sb = consts.tile([P, o_shard], f32)
            nc.scalar.dma_start(out=b_sb[:], in_=b_rep[:])

            W_mm = consts.tile([P, KO, o_shard], bf16)
            for ko in range(KO):
                w_eng = nc.scalar if ko % 2 == 0 else nc.gpsimd
                w_eng.dma_start(out=W_mm[:, ko], in_=Wb_t[:, ko])

            for nch in range(NCH):
                x_ts = []
                for k in range(KO):
                    x_t = xp.tile([P, n_chunk], bf16, tag=f"xt{k}")
                    nc.sync.dma_start(
                        out=x_t[:],
                        in_=xT_t[k, :, nch * n_chunk : (nch + 1) * n_chunk],
                    )
                    x_ts.append(x_t)
                for ns in range(NS):
                    psums = [
                        psump.tile(
                            [P, o_mm], f32,
                            tag=f"ps{ns}_{och}", name=f"ps{ns}_{och}",
                        )
                        for och in range(OCH)
                    ]
                    for k in range(KO):
                        for och in range(OCH):
                            nc.tensor.matmul(
                                psums[och][:],
                                x_ts[k][:, ns * P : (ns + 1) * P],
                                W_mm[:, k, och * o_mm : (och + 1) * o_mm],
                                start=(k == 0),
                                stop=(k == KO - 1),
                            )
                    o_sb = outp.tile([P, o_shard], f32, tag="o")
                    for och in range(OCH):
                        nc.vector.tensor_tensor(
                            o_sb[:, och * o_mm : (och + 1) * o_mm],
                            psums[och][:],
                            b_sb[:, och * o_mm : (och + 1) * o_mm],
                            mybir.AluOpType.add,
                        )
                    row0 = nch * n_chunk + ns * P
                    nc.scalar.dma_start(
                        out=out[row0 : row0 + P, :], in_=o_sb[:]
                    )
    nc.compile()
    return nc


def build_nc_dr(
    n_rows=N_ROWS,
    in_f=IN_F,
    o_shard=O_SHARD,
    n_chunk=512,
    x_bufs=8,
    lo_frac=0.0,
):
    """fp8e4 DoubleRow variant: each matmul consumes a PAIR of k-tiles
    (stationary [128,2,128], moving [128,2,512]) at nominally 0.5 cyc/row.

    lo_frac > 0 adds a correction pass over the first lo_frac of K using a
    second fp8 plane xL = e4m3(x - e4m3(x)), accumulated into the same PSUM
    group, recovering accuracy lost to e4m3 quantization of x.
    """
    f32 = mybir.dt.float32
    fp8 = mybir.dt.float8e4
    DR = mybir.MatmulPerfMode.DoubleRow

    assert in_f % (2 * P) == 0 and n_rows % n_chunk == 0 and n_chunk % P == 0
    OCH = max(1, o_shard // 512)
    o_mm = o_shard // OCH
    assert o_mm <= 512 and o_mm * OCH == o_shard
    KP = in_f // (2 * P)  # k-tile pairs
    KPL = int(round(KP * lo_frac))  # pairs covered by the correction pass
    in_f_lo = KPL * 2 * P
    NCH = n_rows // n_chunk
    NS = n_chunk // P
    psum_bufs = 2 if NS * OCH * 2 <= 8 else 1
    assert NS * OCH * psum_bufs <= 8

    nc = bacc.Bacc("TRN2", target_bir_lowering=False)

    xT = nc.declare_dram_parameter("xT", [in_f, n_rows], fp8, isOutput=False)
    Wb = nc.declare_dram_parameter("Wb", [in_f, o_shard], fp8, isOutput=False)
    a_rep = nc.declare_dram_parameter("a_rep", [P, o_shard], f32, isOutput=False)
    b_rep = nc.declare_dram_parameter("b_rep", [P, o_shard], f32, isOutput=False)
    if KPL:
        xL = nc.declare_dram_parameter(
            "xL", [in_f_lo, n_rows], fp8, isOutput=False
        )
        xL_t = xL[:].rearrange("(kp two p) n -> kp p two n", two=2, p=P)
    out = nc.declare_dram_parameter("out", [n_rows, o_shard], f32, isOutput=True)

    xT_t = xT[:].rearrange("(kp two p) n -> kp p two n", two=2, p=P)
    Wb_t = Wb[:].rearrange("(kp two p) o -> p kp two o", two=2, p=P)

    with tile.TileContext(nc) as tc:
        with (
            tc.tile_pool(name="consts", bufs=1) as consts,
            tc.tile_pool(name="xp", bufs=x_bufs) as xp,
            tc.tile_pool(name="outp", bufs=4) as outp,
            tc.tile_pool(name="psum", bufs=psum_bufs, space="PSUM") as psump,
        ):
            a_sb = consts.tile([P, o_shard], f32)
            nc.scalar.dma_start(out=a_sb[:], in_=a_rep[:])
            b_sb = consts.tile([P, o_shard], f32)
            nc.scalar.dma_start(out=b_sb[:], in_=b_rep[:])

            W_mm = consts.tile([P, KP, 2, o_shard], fp8)
            for kp in range(KP):
                w_eng = nc.scalar if kp % 2 == 0 else nc.gpsimd
                w_eng.dma_start(out=W_mm[:, kp], in_=Wb_t[:, kp])

            n_mm = KP + KPL
            for nch in range(NCH):
                psums = [
                    [
                        psump.tile(
                            [P, o_mm], f32,
                            tag=f"ps{ns}_{och}", name=f"ps{ns}_{och}",
                        )
                        for och in range(OCH)
                    ]
                    for ns in range(NS)
                ]
                mm_i = 0
                for lo in range(2 if KPL else 1):
                    src = xL_t if lo else xT_t
                    for kp in range(KPL if lo else KP):
                        x_t = xp.tile([P, 2, n_chunk], fp8, tag="xt")
                        nc.sync.dma_start(
                            out=x_t[:],
                            in_=src[kp, :, :, nch * n_chunk : (nch + 1) * n_chunk],
                        )
                        for ns in range(NS):
                            for och in range(OCH):
                                nc.tensor.matmul(
                                    psums[ns][och][:],
                                    x_t[:, :, ns * P : (ns + 1) * P],
                                    W_mm[:, kp, :, och * o_mm : (och + 1) * o_mm],
                                    start=(mm_i == 0),
                                    stop=(mm_i == n_mm - 1),
                                    perf_mode=DR,
                                )
                        mm_i += 1
                for ns in range(NS):
                    o_sb = outp.tile([P, o_shard], f32, tag="o")
                    for och in range(OCH):
                        # out = psum * alpha + b  (alpha NOT folded into the
                        # fp8 weights; weights are exact +-1)
                        nc.vector.tensor_tensor(
                            o_sb[:, och * o_mm : (och + 1) * o_mm],
                            psums[ns][och][:],
                            a_sb[:, och * o_mm : (och + 1) * o_mm],
                            mybir.AluOpType.mult,
                        )
                        nc.vector.tensor_tensor(
                            o_sb[:, och * o_mm : (och + 1) * o_mm],
                            o_sb[:, och * o_mm : (och + 1) * o_mm],
                            b_sb[:, och * o_mm : (och + 1) * o_mm],
                            mybir.AluOpType.add,
                        )
                    row0 = nch * n_chunk + ns * P
                    nc.sync.dma_start(
                        out=out[row0 : row0 + P, :], in_=o_sb[:]
                    )
    nc.compile()
    return nc


def build_nc_dr2(
    n_rows=N_ROWS,
    in_f=IN_F,
    o_shard=O_SHARD,
    n_chunk=512,
    kpl=10,
):
    """Tuned fp8e4 DoubleRow variant.

    x is shipped as a hi fp8 plane over all of K plus a lo (residual) fp8
    plane over the first kpl/(in_f/256) fraction of K; both accumulate into
    the same PSUM group, so accuracy ~= e4m3 on the uncorrected tail only.
    Weights are host-binarized to exact +-1 fp8; alpha/bias applied in the
    DVE epilogue.

    Scheduling follows build_nc_hb2: chunk-resident x tiles (per-kp tags,
    double buffered), ns-outer / k-inner matmul loop for pipelined PSUM
    drains, output DMAs on the scalar queue.
    """
    f32 = mybir.dt.float32
    fp8 = mybir.dt.float8e4
    DR = mybir.MatmulPerfMode.DoubleRow

    assert in_f % (2 * P) == 0 and n_rows % n_chunk == 0 and n_chunk % P == 0
    OCH = max(1, o_shard // 512)
    o_mm = o_shard // OCH
    assert o_mm <= 512 and o_mm * OCH == o_shard
    KP = in_f // (2 * P)
    KPL = kpl
    assert 0 <= KPL <= KP
    in_f_lo = KPL * 2 * P
    NCH = n_rows // n_chunk
    NS = n_chunk // P
    psum_bufs = 2 if NS * OCH * 2 <= 8 else 1
    assert NS * OCH * psum_bufs <= 8

    nc = bacc.Bacc("TRN2", target_bir_lowering=False)

    xT = nc.declare_dram_parameter("xT", [in_f, n_rows], fp8, isOutput=False)
    Wb = nc.declare_dram_parameter("Wb", [in_f, o_shard], fp8, isOutput=False)
    a_rep = nc.declare_dram_parameter("a_rep", [P, o_shard], f32, isOutput=False)
    b_rep = nc.declare_dram_parameter("b_rep", [P, o_shard], f32, isOutput=False)
    if KPL:
        xL = nc.declare_dram_parameter(
            "xL", [in_f_lo, n_rows], fp8, isOutput=False
        )
        xL_t = xL[:].rearrange("(kp two p) n -> kp p two n", two=2, p=P)
    out = nc.declare_dram_parameter("out", [n_rows, o_shard], f32, isOutput=True)

    xT_t = xT[:].rearrange("(kp two p) n -> kp p two n", two=2, p=P)
    Wb_t = Wb[:].rearrange("(kp two p) o -> p kp two o", two=2, p=P)

    with tile.TileContext(nc) as tc:
        with (
            tc.tile_pool(name="consts", bufs=1) as consts,
            tc.tile_pool(name="xp", bufs=2) as xp,
            tc.tile_pool(name="outp", bufs=4) as outp,
            tc.tile_pool(name="psum", bufs=psum_bufs, space="PSUM") as psump,
        ):
            W_mm = consts.tile([P, KP, 2, o_shard], fp8)
            for kp in range(KP):
                w_eng = nc.scalar if kp % 2 == 0 else nc.gpsimd
                w_eng.dma_start(out=W_mm[:, kp], in_=Wb_t[:, kp])

            # alpha/bias are first needed at the first epilogue (~8us in);
            # load them after the weights so they don't delay chunk 0
            a_sb = consts.tile([P, o_shard], f32)
            nc.gpsimd.dma_start(out=a_sb[:], in_=a_rep[:])
            b_sb = consts.tile([P, o_shard], f32)
            nc.gpsimd.dma_start(out=b_sb[:], in_=b_rep[:])

            n_mm = KP + KPL
            for nch in range(NCH):
                x_hi, x_lo = [], []
                for kp in range(KP):
                    x_t = xp.tile([P, 2, n_chunk], fp8, tag=f"xh{kp}")
                    nc.sync.dma_start(
                        out=x_t[:],
                        in_=xT_t[kp, :, :, nch * n_chunk : (nch + 1) * n_chunk],
                    )
                    x_hi.append(x_t)
                for kp in range(KPL):
                    x_t = xp.tile([P, 2, n_chunk], fp8, tag=f"xl{kp}")
                    nc.sync.dma_start(
                        out=x_t[:],
                        in_=xL_t[kp, :, :, nch * n_chunk : (nch + 1) * n_chunk],
                    )
                    x_lo.append(x_t)
                for ns in range(NS):
                    psums = [
                        psump.tile(
                            [P, o_mm], f32,
                            tag=f"ps{ns}_{och}", name=f"ps{ns}_{och}",
                        )
                        for och in range(OCH)
                    ]
                    mm_i = 0
                    for tiles in (x_hi, x_lo):
                        for kp, x_t in enumerate(tiles):
                            for och in range(OCH):
                                nc.tensor.matmul(
                                    psums[och][:],
                                    x_t[:, :, ns * P : (ns + 1) * P],
                                    W_mm[:, kp, :, och * o_mm : (och + 1) * o_mm],
                                    start=(mm_i == 0),
                                    stop=(mm_i == n_mm - 1),
                                    perf_mode=DR,
                                )
                            mm_i += 1
                    o_sb = outp.tile([P, o_shard], f32, tag="o")
                    for och in range(OCH):
                        sl = slice(och * o_mm, (och + 1) * o_mm)
                        nc.vector.tensor_tensor(
                            o_sb[:, sl], psums[och][:], a_sb[:, sl],
                            mybir.AluOpType.mult,
                        )
                        nc.vector.tensor_tensor(
                            o_sb[:, sl], o_sb[:, sl], b_sb[:, sl],
                            mybir.AluOpType.add,
                        )
                    row0 = nch * n_chunk + ns * P
                    nc.scalar.dma_start(
                        out=out[row0 : row0 + P, :], in_=o_sb[:]
                    )
    nc.compile()
    return nc


def build_nc_dr3(
    n_rows=N_ROWS,
    in_f=IN_F,
    o_shard=O_SHARD,
    n_chunk=512,
    kpl=10,
    alpha_is_one=False,
    n_warm=12,
):
    """build_nc_dr2 plus startup/teardown tuning:

    - n_warm dummy DoubleRow matmuls on a memset scratch tile fill the
      ~5-10us DMA-queue-init window at kernel start, so the HAM clock gate
      reaches K=8/8 before the first real matmul (saves the half-clock
      ramp that otherwise covers the first ~10us of real work).
    - x tiles are shipped as half-chunks [128, 2, n_chunk/2]; the first
      ns-sweep of a chunk then depends on only half the chunk's bytes, so
      chunk 0 no longer starves a warm PE on a single DMA queue.
    - alpha_is_one=True drops the alpha multiply from the epilogue (the
      caller verifies alpha == 1 on the host): one DVE op per sweep
      instead of two, shortening the end-of-kernel drain chain.
    """
    f32 = mybir.dt.float32
    fp8 = mybir.dt.float8e4
    DR = mybir.MatmulPerfMode.DoubleRow

    assert in_f % (2 * P) == 0 and n_rows % n_chunk == 0 and n_chunk % (2 * P) == 0
    OCH = max(1, o_shard // 512)
    o_mm = o_shard // OCH
    assert o_mm <= 512 and o_mm * OCH == o_shard
    KP = in_f // (2 * P)
    KPL = kpl
    assert 0 <= KPL <= KP
    in_f_lo = KPL * 2 * P
    NCH = n_rows // n_chunk
    NS = n_chunk // P
    NH = n_chunk // 2  # half-chunk width
    psum_bufs = 2 if NS * OCH * 2 <= 8 else 1
    assert NS * OCH * psum_bufs <= 8

    nc = bacc.Bacc("TRN2", target_bir_lowering=False)

    xT = nc.declare_dram_parameter("xT", [in_f, n_rows], fp8, isOutput=False)
    Wb = nc.declare_dram_parameter("Wb", [in_f, o_shard], fp8, isOutput=False)
    if not alpha_is_one:
        a_rep = nc.declare_dram_parameter(
            "a_rep", [P, o_shard], f32, isOutput=False
        )
    b_rep = nc.declare_dram_parameter("b_rep", [P, o_shard], f32, isOutput=False)
    if KPL:
        xL = nc.declare_dram_parameter(
            "xL", [in_f_lo, n_rows], fp8, isOutput=False
        )
        xL_t = xL[:].rearrange("(kp two p) n -> kp p two n", two=2, p=P)
    out = nc.declare_dram_parameter("out", [n_rows, o_shard], f32, isOutput=True)

    xT_t = xT[:].rearrange("(kp two p) n -> kp p two n", two=2, p=P)
    Wb_t = Wb[:].rearrange("(kp two p) o -> p kp two o", two=2, p=P)

    with tile.TileContext(nc) as tc:
        with (
            tc.tile_pool(name="consts", bufs=1) as consts,
            tc.tile_pool(name="xp", bufs=2) as xp,
            tc.tile_pool(name="outp", bufs=4) as outp,
            tc.tile_pool(name="psum", bufs=psum_bufs, space="PSUM") as psump,
        ):
            # PE warm-up: dummy DR matmuls on zeroed scratch while the DMA
            # queues initialize and chunk 0 streams in.
            if n_warm:
                warm = consts.tile([P, 2, 512], fp8)
                nc.vector.memset(warm[:], 0.0)
                wps = psump.tile([P, o_mm], f32, tag="ps0_0", name="warm")
                for _ in range(n_warm):
                    nc.tensor.matmul(
                        wps[:], warm[:, :, :P], warm[:, :, :o_mm],
                        start=True, stop=True, perf_mode=DR,
                    )

            W_mm = consts.tile([P, KP, 2, o_shard], fp8)
            for kp in range(KP):
                w_eng = nc.scalar if kp % 2 == 0 else nc.gpsimd
                w_eng.dma_start(out=W_mm[:, kp], in_=Wb_t[:, kp])

            if not alpha_is_one:
                a_sb = consts.tile([P, o_shard], f32)
                nc.gpsimd.dma_start(out=a_sb[:], in_=a_rep[:])
            b_sb = consts.tile([P, o_shard], f32)
            nc.gpsimd.dma_start(out=b_sb[:], in_=b_rep[:])

            n_mm = KP + KPL
            for nch in range(NCH):
                x_hi = [[], []]
                x_lo = [[], []]
                for h in range(2):
                    col0 = nch * n_chunk + h * NH
                    for kp in range(KP):
                        x_t = xp.tile([P, 2, NH], fp8, tag=f"xh{h}_{kp}")
                        nc.sync.dma_start(
                            out=x_t[:], in_=xT_t[kp, :, :, col0 : col0 + NH]
                        )
                        x_hi[h].append(x_t)
                    for kp in range(KPL):
                        x_t = xp.tile([P, 2, NH], fp8, tag=f"xl{h}_{kp}")
                        nc.sync.dma_start(
                            out=x_t[:], in_=xL_t[kp, :, :, col0 : col0 + NH]
                        )
                        x_lo[h].append(x_t)
                for ns in range(NS):
                    h, hs = divmod(ns, NH // P)
                    psums = [
                        psump.tile(
                            [P, o_mm], f32,
                            tag=f"ps{ns}_{och}", name=f"ps{ns}_{och}",
                        )
                        for och in range(OCH)
                    ]
                    mm_i = 0
                    for tiles in (x_hi[h], x_lo[h]):
                        for kp, x_t in enumerate(tiles):
                            for och in range(OCH):
                                nc.tensor.matmul(
                                    psums[och][:],
                                    x_t[:, :, hs * P : (hs + 1) * P],
                                    W_mm[:, kp, :, och * o_mm : (och + 1) * o_mm],
                                    start=(mm_i == 0),
                                    stop=(mm_i == n_mm - 1),
                                    perf_mode=DR,
                                )
                            mm_i += 1
                    o_sb = outp.tile([P, o_shard], f32, tag="o")
                    for och in range(OCH):
                        sl = slice(och * o_mm, (och + 1) * o_mm)
                        if alpha_is_one:
                            nc.vector.tensor_tensor(
                                o_sb[:, sl], psums[och][:], b_sb[:, sl],
                                mybir.AluOpType.add,
                            )
                        else:
                            nc.vector.tensor_tensor(
                                o_sb[:, sl], psums[och][:], a_sb[:, sl],
                                mybir.AluOpType.mult,
                            )
                            nc.vector.tensor_tensor(
                                o_sb[:, sl], o_sb[:, sl], b_sb[:, sl],
                                mybir.AluOpType.add,
                            )
                    row0 = nch * n_chunk + ns * P
                    nc.scalar.dma_start(
                        out=out[row0 : row0 + P, :], in_=o_sb[:]
                    )
    nc.compile()
    return nc


def build_nc_dr4(
    n_rows=N_ROWS,
    in_f=IN_F,
    o_shard=O_SHARD,
    n_chunk=512,
    kpl=10,
    alpha_is_one=False,
    n_warm=12,
):
    """build_nc_dr2 + warm-up + fused epilogue + HOST-PRE-TILED x/W.

    The x stream's DMA queue is packet-rate limited (~330 packets/us); with
    the natural [in_f, n] layout each [128,2,n_chunk] tile costs 256
    packets of 512B (the DoubleRow pair dim breaks contiguity). Here the
    host ships x already tiled as [KP][NCH][128][2*n_chunk] so every
    partition line is one contiguous 1KB packet: 128 packets/tile, halving
    the queue's packet load (~160us floor vs ~343us), which removes all
    x-stream starvation. Same for the (small) weight tensor.

    Also: n_warm dummy DR matmuls on zeroed scratch warm the HAM clock
    gate during queue init; chunk 0's lo tiles ride the gpsimd queue so
    the first sweep isn't single-queue bound; alpha_is_one drops the
    epilogue multiply (caller checks alpha==1 on host).
    """
    f32 = mybir.dt.float32
    fp8 = mybir.dt.float8e4
    DR = mybir.MatmulPerfMode.DoubleRow

    assert in_f % (2 * P) == 0 and n_rows % n_chunk == 0 and n_chunk % P == 0
    OCH = max(1, o_shard // 512)
    o_mm = o_shard // OCH
    assert o_mm <= 512 and o_mm * OCH == o_shard
    KP = in_f // (2 * P)
    KPL = kpl
    assert 0 <= KPL <= KP
    NCH = n_rows // n_chunk
    NS = n_chunk // P
    psum_bufs = 2 if NS * OCH * 2 <= 8 else 1
    assert NS * OCH * psum_bufs <= 8

    nc = bacc.Bacc("TRN2", target_bir_lowering=False)

    xT = nc.declare_dram_parameter(
        "xT", [KP, NCH, P, 2, n_chunk], fp8, isOutput=False
    )
    Wb = nc.declare_dram_parameter(
        "Wb", [KP, P, 2, o_shard], fp8, isOutput=False
    )
    if not alpha_is_one:
        a_rep = nc.declare_dram_parameter(
            "a_rep", [P, o_shard], f32, isOutput=False
        )
    b_rep = nc.declare_dram_parameter("b_rep", [P, o_shard], f32, isOutput=False)
    if KPL:
        xL = nc.declare_dram_parameter(
            "xL", [KPL, NCH, P, 2, n_chunk], fp8, isOutput=False
        )
    out = nc.declare_dram_parameter("out", [n_rows, o_shard], f32, isOutput=True)

    with tile.TileContext(nc) as tc:
        with (
            tc.tile_pool(name="consts", bufs=1) as consts,
            tc.tile_pool(name="xp", bufs=2) as xp,
            tc.tile_pool(name="outp", bufs=4) as outp,
            tc.tile_pool(name="psum", bufs=psum_bufs, space="PSUM") as psump,
        ):
            if n_warm:
                warm = consts.tile([P, 2, 512], fp8)
                nc.vector.memset(warm[:], 0.0)
                wps = psump.tile([P, o_mm], f32, tag="ps0_0", name="warm")
                for _ in range(n_warm):
                    nc.tensor.matmul(
                        wps[:], warm[:, :, :P], warm[:, :, :o_mm],
                        start=True, stop=True, perf_mode=DR,
                    )

            # dedicated queues during the cold window: W + alpha/bias on
            # scalar (done ~15us, before the first output DMA), lo plane on
            # gpsimd from t=0, hi plane alone on sync
            W_mm = consts.tile([P, KP, 2, o_shard], fp8)
            for kp in range(KP):
                nc.scalar.dma_start(out=W_mm[:, kp], in_=Wb[kp])

            if not alpha_is_one:
                a_sb = consts.tile([P, o_shard], f32)
                nc.scalar.dma_start(out=a_sb[:], in_=a_rep[:])
            b_sb = consts.tile([P, o_shard], f32)
            nc.scalar.dma_start(out=b_sb[:], in_=b_rep[:])

            n_mm = KP + KPL
            for nch in range(NCH):
                x_hi, x_lo = [], []
                for kp in range(KP):
                    x_t = xp.tile([P, 2, n_chunk], fp8, tag=f"xh{kp}")
                    nc.sync.dma_start(out=x_t[:], in_=xT[kp, nch])
                    x_hi.append(x_t)
                for kp in range(KPL):
                    x_t = xp.tile([P, 2, n_chunk], fp8, tag=f"xl{kp}")
                    # lo tiles ride gpsimd (idle after the weight load):
                    # splits the x stream over two queues so neither the
                    # cold-start chunks nor steady state are queue-bound
                    nc.gpsimd.dma_start(out=x_t[:], in_=xL[kp, nch])
                    x_lo.append(x_t)
                for ns in range(NS):
                    psums = [
                        psump.tile(
                            [P, o_mm], f32,
                            tag=f"ps{ns}_{och}", name=f"ps{ns}_{och}",
                        )
                        for och in range(OCH)
                    ]
                    mm_i = 0
                    for tiles in (x_hi, x_lo):
                        for kp, x_t in enumerate(tiles):
                            for och in range(OCH):
                                nc.tensor.matmul(
                                    psums[och][:],
                                    x_t[:, :, ns * P : (ns + 1) * P],
                                    W_mm[:, kp, :, och * o_mm : (och + 1) * o_mm],
                                    start=(mm_i == 0),
                                    stop=(mm_i == n_mm - 1),
                                    perf_mode=DR,
                                )
                            mm_i += 1
                    o_sb = outp.tile([P, o_shard], f32, tag="o")
                    # epilogue must be on DVE: it is the only engine that
                    # can read PSUM besides Activation (gpsimd cannot)
                    e_eng = nc.vector
                    for och in range(OCH):
                        sl = slice(och * o_mm, (och + 1) * o_mm)
                        if alpha_is_one:
                            e_eng.tensor_tensor(
                                o_sb[:, sl], psums[och][:], b_sb[:, sl],
                                mybir.AluOpType.add,
                            )
                        else:
                            e_eng.tensor_tensor(
                                o_sb[:, sl], psums[och][:], a_sb[:, sl],
                                mybir.AluOpType.mult,
                            )
                            e_eng.tensor_tensor(
                                o_sb[:, sl], o_sb[:, sl], b_sb[:, sl],
                                mybir.AluOpType.add,
                            )
                    row0 = nch * n_chunk + ns * P
                    nc.scalar.dma_start(
                        out=out[row0 : row0 + P, :], in_=o_sb[:]
                    )
    nc.compile()
    return nc


def make_in_maps_dr4(
    x, W, alpha, b, n_cores=N_CORES, grid=(1, 8), lo_frac=0.0,
    n_chunk=512, alpha_is_one=False,
):
    """Host-side sharding + pre-tiling into the DoubleRow DMA layout."""
    import ml_dtypes

    e4 = ml_dtypes.float8_e4m3
    xs, ws = grid
    assert xs * ws == n_cores
    n_shard = x.shape[0] // xs
    o_shard = W.shape[0] // ws
    in_f = x.shape[1]
    KP = in_f // (2 * P)
    KPL = int(round(KP * lo_frac))
    in_f_lo = KPL * 2 * P
    NCH = n_shard // n_chunk

    xT32 = np.ascontiguousarray(x.T)
    xT = xT32.astype(e4)

    def tile5(a):  # [in_f_part, n_shard] -> [KP', NCH, P, 2, n_chunk]
        kp = a.shape[0] // (2 * P)
        return np.ascontiguousarray(
            a.reshape(kp, 2, P, NCH, n_chunk).transpose(0, 3, 2, 1, 4)
        )

    x_parts = [
        tile5(xT[:, r * n_shard : (r + 1) * n_shard]) for r in range(xs)
    ]
    if KPL:
        xL32 = xT32[:in_f_lo] - xT[:in_f_lo].astype(np.float32)
        xLf = xL32.astype(e4)
        xl_parts = [
            tile5(xLf[:, r * n_shard : (r + 1) * n_shard]) for r in range(xs)
        ]
    sgn = np.where(W >= 0, np.float32(1.0), np.float32(-1.0)).astype(e4)
    w_parts = {}
    in_maps = []
    for c in range(n_cores):
        r, q = divmod(c, ws)
        if q not in w_parts:
            sl = slice(q * o_shard, (q + 1) * o_shard)
            wT = np.ascontiguousarray(sgn[sl].T)  # [in_f, o_shard]
            w_parts[q] = {
                "Wb": np.ascontiguousarray(
                    wT.reshape(KP, 2, P, o_shard).transpose(0, 2, 1, 3)
                ),
                "b_rep": np.ascontiguousarray(
                    np.broadcast_to(
                        b[sl].reshape(1, -1).astype(np.float32), (P, o_shard)
                    )
                ),
            }
            if not alpha_is_one:
                w_parts[q]["a_rep"] = np.ascontiguousarray(
                    np.broadcast_to(
                        alpha[sl].reshape(1, -1).astype(np.float32),
                        (P, o_shard),
                    )
                )
        m = {"xT": x_parts[r], **w_parts[q]}
        if KPL:
            m["xL"] = xl_parts[r]
        in_maps.append(m)
    return in_maps


def make_in_maps_dr(x, W, alpha, b, n_cores=N_CORES, grid=(1, 8), lo_frac=0.0):
    import ml_dtypes

    e4 = ml_dtypes.float8_e4m3
    xs, ws = grid
    assert xs * ws == n_cores
    n_shard = x.shape[0] // xs
    o_shard = W.shape[0] // ws
    xT32 = np.ascontiguousarray(x.T)
    xT = xT32.astype(e4)
    in_f = x.shape[1]
    KP = in_f // (2 * P)
    KPL = int(round(KP * lo_frac))
    in_f_lo = KPL * 2 * P
    x_parts = [
        np.ascontiguousarray(xT[:, r * n_shard : (r + 1) * n_shard])
        for r in range(xs)
    ]
    if KPL:
        xL32 = xT32[:in_f_lo] - xT[:in_f_lo].astype(np.float32)
        xLf = xL32.astype(e4)
        xl_parts = [
            np.ascontiguousarray(xLf[:, r * n_shard : (r + 1) * n_shard])
            for r in range(xs)
        ]
    sgn = np.where(W >= 0, np.float32(1.0), np.float32(-1.0)).astype(e4)
    w_parts = {}
    in_maps = []
    for c in range(n_cores):
        r, q = divmod(c, ws)
        if q not in w_parts:
            sl = slice(q * o_shard, (q + 1) * o_shard)
            w_parts[q] = {
                "Wb": np.ascontiguousarray(sgn[sl].T),
                "a_rep": np.ascontiguousarray(
                    np.broadcast_to(
                        alpha[sl].reshape(1, -1).astype(np.float32),
                        (P, o_shard),
                    )
                ),
                "b_rep": np.ascontiguousarray(
                    np.broadcast_to(
                        b[sl].reshape(1, -1).astype(np.float32), (P, o_shard)
                    )
                ),
            }
        m = {"xT": x_parts[r], **w_parts[q]}
        if KPL:
            m["xL"] = xl_parts[r]
        in_maps.append(m)
    return in_maps


def make_in_maps_hb(x, W, alpha, b, n_cores=N_CORES, grid=(1, 8)):
    """Shard full inputs into per-core input maps (host-side only).

    Weights are binarized here: Wb = bf16(sign(W)) * bf16(alpha), matching
    the reference's sign(W)*alpha then the matmul-input bf16 rounding.
    """
    import ml_dtypes

    bf16 = ml_dtypes.bfloat16
    xs, ws = grid
    assert xs * ws == n_cores
    n_shard = x.shape[0] // xs
    o_shard = W.shape[0] // ws
    xT = np.ascontiguousarray(x.T).astype(bf16)
    x_parts = [
        np.ascontiguousarray(xT[:, r * n_shard : (r + 1) * n_shard])
        for r in range(xs)
    ]
    # sign in f32 (exact), multiply by alpha in f32, round once to bf16
    sgn = np.where(W >= 0, np.float32(1.0), np.float32(-1.0))
    bw = (sgn * alpha).astype(bf16)  # [out, in]
    w_parts = {}
    in_maps = []
    for c in range(n_cores):
        r, q = divmod(c, ws)
        if q not in w_parts:
            sl = slice(q * o_shard, (q + 1) * o_shard)
            w_parts[q] = {
                "Wb": np.ascontiguousarray(bw[sl].T),
                "b_rep": np.ascontiguousarray(
                    np.broadcast_to(
                        b[sl].reshape(1, -1).astype(np.float32), (P, o_shard)
                    )
                ),
            }
        in_maps.append({"xT": x_parts[r], **w_parts[q]})
    return in_maps


_NC_CACHE = {}


def kernel(x, W, alpha, b, trace=False, variant=VARIANT):
    x = np.asarray(x, dtype=np.float32)
    W = np.asarray(W, dtype=np.float32)
    alpha = np.asarray(alpha, dtype=np.float32)
    b = np.asarray(b, dtype=np.float32)

    n_rows, in_f = x.shape
    out_f = W.shape[0]
    grid = (1, 8)
    xs, ws = grid
    n_shard = n_rows // xs
    o_shard = out_f // ws

    # drE: dr3 (warm-up + half-tiles + fused epilogue when alpha == 1).
    # drA/B/C: dr2 with 10/11/12 of 16 k-pairs corrected.
    # dr1/dr15/dr2: first-cut DoubleRow probes.
    KPL_OF = {
        "drF": 10, "drE": 10, "drA": 10, "drB": 11, "drC": 12,
        "dr1": 0, "dr15": 8, "dr2": 16,
    }
    lo_frac = KPL_OF[variant] / 16.0 if variant in KPL_OF else 0.0
    alpha_is_one = bool(np.all(alpha == 1.0))

    key = (n_rows, in_f, variant, alpha_is_one)
    if key not in _NC_CACHE:
        if variant == "drF":
            _NC_CACHE[key] = build_nc_dr4(
                n_rows=n_shard, in_f=in_f, o_shard=o_shard,
                kpl=KPL_OF[variant], alpha_is_one=alpha_is_one,
            )
        elif variant == "drE":
            _NC_CACHE[key] = build_nc_dr3(
                n_rows=n_shard, in_f=in_f, o_shard=o_shard,
                kpl=KPL_OF[variant], alpha_is_one=alpha_is_one,
            )
        elif variant in ("drA", "drB", "drC"):
            _NC_CACHE[key] = build_nc_dr2(
                n_rows=n_shard, in_f=in_f, o_shard=o_shard,
                kpl=KPL_OF[variant],
            )
        elif variant.startswith("dr"):
            _NC_CACHE[key] = build_nc_dr(
                n_rows=n_shard, in_f=in_f, o_shard=o_shard, lo_frac=lo_frac
            )
        elif variant == "hb2":
            _NC_CACHE[key] = build_nc_hb2(
                n_rows=n_shard, in_f=in_f, o_shard=o_shard
            )
        else:
            _NC_CACHE[key] = build_nc_hb(
                n_rows=n_shard, in_f=in_f, o_shard=o_shard
            )
    nc = _NC_CACHE[key]

    if variant == "drF":
        in_maps = make_in_maps_dr4(
            x, W, alpha, b, grid=grid, lo_frac=lo_frac,
            alpha_is_one=alpha_is_one,
        )
    elif variant.startswith("dr"):
        in_maps = make_in_maps_dr(x, W, alpha, b, grid=grid, lo_frac=lo_frac)
        if variant == "drE" and alpha_is_one:
            in_maps = [
                {k: v for k, v in m.items() if k != "a_rep"} for m in in_maps
            ]
    else:
        in_maps = make_in_maps_hb(x, W, alpha, b, grid=grid)
    try:
        res = run_bass_kernel_spmd(
            nc, in_maps, core_ids=list(range(N_CORES)), trace=trace
        )
    except Exception:
        # The trace path needs antenv.axon_hooks + artifact upload, which
        # some containers lack. If we didn't ask for tracing ourselves,
        # retry once with tracing force-disabled instead of failing.
        if trace:
            raise
        os.environ["BASS_NEVER_TRACE"] = "1"
        res = run_bass_kernel_spmd(
            nc, in_maps, core_ids=list(range(N_CORES)), trace=False
        )
    full = np.empty((n_rows, out_f), dtype=np.float32)
    for c in range(N_CORES):
        r, q = divmod(c, ws)
        full[
            r * n_shard : (r + 1) * n_shard, q * o_shard : (q + 1) * o_shard
        ] = np.asarray(res.results[c]["out"])
    if trace:
        return full, res
    return full


if __name__ == "__main__":
    nc = build_nc_hb(n_rows=512, in_f=512, o_shard=256, n_chunk=256)
    print("build ok [hb]")


# revision 28
# speedup vs baseline: 1.0054x; 1.0054x over previous
"""BinaryLinear (8192x4096 @ 4096x4096 binarized) on 8 TRN2 NeuronCores.

Strategy (tensor-parallel, column sharding per out_features):
  - Shard W/alpha/b along out_features: each core gets 512 output channels.
  - Replicate x (host pre-transposed to [in_f, n_rows] so the contraction
    dim lands on SBUF partitions without any device-side transpose).
  - Host gathers the 8 [8192, 512] shards with a concatenate on axis 1.

Default variant — fp8e4 DoubleRow (measured ~387us vs 503us bf16
baseline):
  - The binarized weights sign(W) are EXACT in fp8 (+-1); alpha and bias
    are applied in the f32 DVE epilogue, so weight precision is perfect.
  - x is shipped as a hi e4m3 plane over all of K plus a residual e4m3
    plane over the first 10/16 of K, both accumulated into the same PSUM
    group. fp8 products are exact on the PE (e6m3 upcast, e10m10 products)
    so the only error is e4m3 quantization of x on the UNcorrected 6/16 of
    K: rel err = 2.654e-2 * sqrt(6/16) = 1.626e-2 < 2e-2 gate, verified
    bit-exact against numpy on hardware.
  - DoubleRow perf mode packs 2 fp8 weights per PE cell: each matmul
    consumes a PAIR of k-tiles (stationary [128,2,128], moving [128,2,512])
    in 512 cycles -> 2x bf16 MAC throughput (216ns/MM measured). Total
    stream = (16 hi + 10 lo) pairs x 64 row-blocks = 1664 MMs ~ 359us.
  - Scheduling: whole n-chunk of x tiles resident in SBUF (per-kp tags,
    double buffered), ns-outer/k-inner loop so PSUM drains pipeline with
    compute; 8 PSUM banks via per-ns tags; weights on scalar+gpsimd DMA
    queues, x stream on sync queue, outputs on scalar queue.
  - drF additionally ships x/W pre-tiled on the host so each DMA
    partition line is one contiguous 1KB packet (the x queue is
    packet-rate limited at ~330 packets/us; natural layout was within 5%
    of that limit), warms the HAM clock gate with dummy matmuls on zeroed
    scratch during queue init, and drops the alpha multiply from the
    epilogue when alpha == 1 (checked on the host; general path kept).

Fallback variants kept for reference: drA (same math, natural layout),
hb/hb2 (host-binarized bf16, ~455-485us, rel err 1.7e-3),
dr1/dr15/dr2/drB/drC (other fp8 splits).
"""

import os
import sys

sys.path.insert(0, "/opt/trn_rl_repo")

import numpy as np

from concourse import bacc, bass, mybir
import concourse.tile as tile
from concourse.bass_utils import run_bass_kernel_spmd

N_ROWS = 8192
IN_F = 4096
OUT_F = 4096
N_CORES = 8
O_SHARD = OUT_F // N_CORES  # 512

P = 128

VARIANT = "drF"  # fp8 DoubleRow, 10/16 residual correction, pre-tiled DMA


def build_nc_hb(
    n_rows=N_ROWS,
    in_f=IN_F,
    o_shard=O_SHARD,
    n_chunk=512,
    x_bufs=8,
):
    """Per-core Bass graph, host-binarized bf16 weights (SPMD on all cores)."""
    f32 = mybir.dt.float32
    bf16 = mybir.dt.bfloat16

    assert in_f % P == 0 and n_rows % n_chunk == 0 and n_chunk % P == 0
    OCH = max(1, o_shard // 512)  # 512-wide o-chunks (one PSUM bank each)
    o_mm = o_shard // OCH
    assert o_mm <= 512 and o_mm * OCH == o_shard
    KO = in_f // P
    NCH = n_rows // n_chunk
    NS = n_chunk // P
    psum_bufs = 2 if NS * OCH * 2 <= 8 else 1
    assert NS * OCH * psum_bufs <= 8

    nc = bacc.Bacc("TRN2", target_bir_lowering=False)

    xT = nc.declare_dram_parameter("xT", [in_f, n_rows], bf16, isOutput=False)
    Wb = nc.declare_dram_parameter("Wb", [in_f, o_shard], bf16, isOutput=False)
    b_rep = nc.declare_dram_parameter("b_rep", [P, o_shard], f32, isOutput=False)
    out = nc.declare_dram_parameter("out", [n_rows, o_shard], f32, isOutput=True)

    xT_t = xT[:].rearrange("(ko p) n -> ko p n", p=P)
    Wb_t = Wb[:].rearrange("(ko p) o -> p ko o", p=P)

    with tile.TileContext(nc) as tc:
        with (
            tc.tile_pool(name="consts", bufs=1) as consts,
            tc.tile_pool(name="xp", bufs=x_bufs) as xp,
            tc.tile_pool(name="outp", bufs=4) as outp,
            tc.tile_pool(name="psum", bufs=psum_bufs, space="PSUM") as psump,
        ):
            # Weight/bias loads go through the scalar+gpsimd HWDGE queues so
            # the x-tile stream (sync queue) isn't stuck behind them.
            b_sb = consts.tile([P, o_shard], f32)
            nc.scalar.dma_start(out=b_sb[:], in_=b_rep[:])

            W_mm = consts.tile([P, KO, o_shard], bf16)
            for ko in range(KO):
                w_eng = nc.scalar if ko % 2 == 0 else nc.gpsimd
                w_eng.dma_start(out=W_mm[:, ko], in_=Wb_t[:, ko])

            for nch in range(NCH):
                psums = [
                    [
                        psump.tile(
                            [P, o_mm], f32,
                            tag=f"ps{ns}_{och}", name=f"ps{ns}_{och}",
                        )
                        for och in range(OCH)
                    ]
                    for ns in range(NS)
                ]
                for k in range(KO):
                    x_t = xp.tile([P, n_chunk], bf16, tag="xt")
                    nc.sync.dma_start(
                        out=x_t[:],
                        in_=xT_t[k, :, nch * n_chunk : (nch + 1) * n_chunk],
                    )
                    for ns in range(NS):
                        for och in range(OCH):
                            nc.tensor.matmul(
                                psums[ns][och][:],
                                x_t[:, ns * P : (ns + 1) * P],
                                W_mm[:, k, och * o_mm : (och + 1) * o_mm],
                                start=(k == 0),
                                stop=(k == KO - 1),
                            )
                for ns in range(NS):
                    o_sb = outp.tile([P, o_shard], f32, tag="o")
                    for och in range(OCH):
                        nc.vector.tensor_tensor(
                            o_sb[:, och * o_mm : (och + 1) * o_mm],
                            psums[ns][och][:],
                            b_sb[:, och * o_mm : (och + 1) * o_mm],
                            mybir.AluOpType.add,
                        )
                    row0 = nch * n_chunk + ns * P
                    nc.sync.dma_start(
                        out=out[row0 : row0 + P, :], in_=o_sb[:]
                    )
    nc.compile()
    return nc


def build_nc_hb2(
    n_rows=N_ROWS,
    in_f=IN_F,
    o_shard=O_SHARD,
    n_chunk=512,
):
    """Tuned host-binarized bf16 variant.

    vs build_nc_hb:
      - x tiles for a whole n-chunk stay resident in SBUF (per-k tags,
        double-buffered across chunks) and the matmul loop is ns-outer /
        k-inner, so each PSUM group finishes ~n_chunk/P times earlier and
        its DVE epilogue + output DMA overlap the next group's matmuls
        (shrinks the end-of-kernel drain tail).
      - output DMAs go on the scalar queue (idle after the weight load)
        instead of the sync queue, so they never delay the x-tile stream
        that feeds LDWEIGHTS at chunk boundaries.
    """
    f32 = mybir.dt.float32
    bf16 = mybir.dt.bfloat16

    assert in_f % P == 0 and n_rows % n_chunk == 0 and n_chunk % P == 0
    OCH = max(1, o_shard // 512)
    o_mm = o_shard // OCH
    assert o_mm <= 512 and o_mm * OCH == o_shard
    KO = in_f // P
    NCH = n_rows // n_chunk
    NS = n_chunk // P
    psum_bufs = 2 if NS * OCH * 2 <= 8 else 1
    assert NS * OCH * psum_bufs <= 8

    nc = bacc.Bacc("TRN2", target_bir_lowering=False)

    xT = nc.declare_dram_parameter("xT", [in_f, n_rows], bf16, isOutput=False)
    Wb = nc.declare_dram_parameter("Wb", [in_f, o_shard], bf16, isOutput=False)
    b_rep = nc.declare_dram_parameter("b_rep", [P, o_shard], f32, isOutput=False)
    out = nc.declare_dram_parameter("out", [n_rows, o_shard], f32, isOutput=True)

    xT_t = xT[:].rearrange("(ko p) n -> ko p n", p=P)
    Wb_t = Wb[:].rearrange("(ko p) o -> p ko o", p=P)

    with tile.TileContext(nc) as tc:
        with (
            tc.tile_pool(name="consts", bufs=1) as consts,
            tc.tile_pool(name="xp", bufs=2) as xp,
            tc.tile_pool(name="outp", bufs=4) as outp,
            tc.tile_pool(name="psum", bufs=psum_bufs, space="PSUM") as psump,
        ):
            b_sb = consts.tile([P, o_shard], f32)
            nc.scalar.dma_start(out=b_sb[:], in_=b_rep[:])

            W_mm = consts.tile([P, KO, o_shard], bf16)
            for ko in range(KO):
                w_eng = nc.scalar if ko % 2 == 0 else nc.gpsimd
                w_eng.dma_start(out=W_mm[:, ko], in_=Wb_t[:, ko])

            for nch in range(NCH):
                x_ts = []
                for k in range(KO):
                    x_t = xp.tile([P, n_chunk], bf16, tag=f"xt{k}")
                    nc.sync.dma_start(
                        out=x_t[:],
                        in_=xT_t[k, :, nch * n_chunk : (nch + 1) * n_chunk],
                    )
                    x_ts.append(x_t)
                for ns in range(NS):
                    psums = [
                        psump.tile(
                            [P, o_mm], f32,
                            tag=f"ps{ns}_{och}", name=f"ps{ns}_{och}",
                        )
                        for och in range(OCH)
                    ]
                    for k in range(KO):
                        for och in range(OCH):
                            nc.tensor.matmul(
                                psums[och][:],
                                x_ts[k][:, ns * P : (ns + 1) * P],
                                W_mm[:, k, och * o_mm : (och + 1) * o_mm],
                                start=(k == 0),
                                stop=(k == KO - 1),
                            )
                    o_sb = outp.tile([P, o_shard], f32, tag="o")
                    for och in range(OCH):
                        nc.vector.tensor_tensor(
                            o_sb[:, och * o_mm : (och + 1) * o_mm],
                            psums[och][:],
                            b_sb[:, och * o_mm : (och + 1) * o_mm],
                            mybir.AluOpType.add,
                        )
                    row0 = nch * n_chunk + ns * P
                    nc.scalar.dma_start(
                        out=out[row0 : row0 + P, :], in_=o_sb[:]
                    )
    nc.compile()
    return nc


def build_nc_dr(
    n_rows=N_ROWS,
    in_f=IN_F,
    o_shard=O_SHARD,
    n_chunk=512,
    x_bufs=8,
    lo_frac=0.0,
):
    """fp8e4 DoubleRow variant: each matmul consumes a PAIR of k-tiles
    (stationary [128,2,128], moving [128,2,512]) at nominally 0.5 cyc/row.

    lo_frac > 0 adds a correction pass over the first lo_frac of K using a
    second fp8 plane xL = e4m3(x - e4m3(x)), accumulated into the same PSUM
    group, recovering accuracy lost to e4m3 quantization of x.
    """
    f32 = mybir.dt.float32
    fp8 = mybir.dt.float8e4
    DR = mybir.MatmulPerfMode.DoubleRow

    assert in_f % (2 * P) == 0 and n_rows % n_chunk == 0 and n_chunk % P == 0
    OCH = max(1, o_shard // 512)
    o_mm = o_shard // OCH
    assert o_mm <= 512 and o_mm * OCH == o_shard
    KP = in_f // (2 * P)  # k-tile pairs
    KPL = int(round(KP * lo_frac))  # pairs covered by the correction pass
    in_f_lo = KPL * 2 * P
    NCH = n_rows // n_chunk
    NS = n_chunk // P
    psum_bufs = 2 if NS * OCH * 2 <= 8 else 1
    assert NS * OCH * psum_bufs <= 8

    nc = bacc.Bacc("TRN2", target_bir_lowering=False)

    xT = nc.declare_dram_parameter("xT", [in_f, n_rows], fp8, isOutput=False)
    Wb = nc.declare_dram_parameter("Wb", [in_f, o_shard], fp8, isOutput=False)
    a_rep = nc.declare_dram_parameter("a_rep", [P, o_shard], f32, isOutput=False)
    b_rep = nc.declare_dram_parameter("b_rep", [P, o_shard], f32, isOutput=False)
    if KPL:
        xL = nc.declare_dram_parameter(
            "xL", [in_f_lo, n_rows], fp8, isOutput=False
        )
        xL_t = xL[:].rearrange("(kp two p) n -> kp p two n", two=2, p=P)
    out = nc.declare_dram_parameter("out", [n_rows, o_shard], f32, isOutput=True)

    xT_t = xT[:].rearrange("(kp two p) n -> kp p two n", two=2, p=P)
    Wb_t = Wb[:].rearrange("(kp two p) o -> p kp two o", two=2, p=P)

    with tile.TileContext(nc) as tc:
        with (
            tc.tile_pool(name="consts", bufs=1) as consts,
            tc.tile_pool(name="xp", bufs=x_bufs) as xp,
            tc.tile_pool(name="outp", bufs=4) as outp,
            tc.tile_pool(name="psum", bufs=psum_bufs, space="PSUM") as psump,
        ):
            a_sb = consts.tile([P, o_shard], f32)
            nc.scalar.dma_start(out=a_sb[:], in_=a_rep[:])
            b_sb = consts.tile([P, o_shard], f32)
            nc.scalar.dma_start(out=b_sb[:], in_=b_rep[:])

            W_mm = consts.tile([P, KP, 2, o_shard], fp8)
            for kp in range(KP):
                w_eng = nc.scalar if kp % 2 == 0 else nc.gpsimd
                w_eng.dma_start(out=W_mm[:, kp], in_=Wb_t[:, kp])

            n_mm = KP + KPL
            for nch in range(NCH):
                psums = [
                    [
                        psump.tile(
                            [P, o_mm], f32,
                            tag=f"ps{ns}_{och}", name=f"ps{ns}_{och}",
                        )
                        for och in range(OCH)
                    ]
                    for ns in range(NS)
                ]
                mm_i = 0
                for lo in range(2 if KPL else 1):
                    src = xL_t if lo else xT_t
                    for kp in range(KPL if lo else KP):
                        x_t = xp.tile([P, 2, n_chunk], fp8, tag="xt")
                        nc.sync.dma_start(
                            out=x_t[:],
                            in_=src[kp, :, :, nch * n_chunk : (nch + 1) * n_chunk],
                        )
                        for ns in range(NS):
                            for och in range(OCH):
                                nc.tensor.matmul(
                                    psums[ns][och][:],
                                    x_t[:, :, ns * P : (ns + 1) * P],
                                    W_mm[:, kp, :, och * o_mm : (och + 1) * o_mm],
                                    start=(mm_i == 0),
                                    stop=(mm_i == n_mm - 1),
                                    perf_mode=DR,
                                )
                        mm_i += 1
                for ns in range(NS):
                    o_sb = outp.tile([P, o_shard], f32, tag="o")
                    for och in range(OCH):
                        # out = psum * alpha + b  (alpha NOT folded into the
                        # fp8 weights; weights are exact +-1)
                        nc.vector.tensor_tensor(
                            o_sb[:, och * o_mm : (och + 1) * o_mm],
                            psums[ns][och][:],
                            a_sb[:, och * o_mm : (och + 1) * o_mm],
                            mybir.AluOpType.mult,
                        )
                        nc.vector.tensor_tensor(
                            o_sb[:, och * o_mm : (och + 1) * o_mm],
                            o_sb[:, och * o_mm : (och + 1) * o_mm],
                            b_sb[:, och * o_mm : (och + 1) * o_mm],
                            mybir.AluOpType.add,
                        )
                    row0 = nch * n_chunk + ns * P
                    nc.sync.dma_start(
                        out=out[row0 : row0 + P, :], in_=o_sb[:]
                    )
    nc.compile()
    return nc


def build_nc_dr2(
    n_rows=N_ROWS,
    in_f=IN_F,
    o_shard=O_SHARD,
    n_chunk=512,
    kpl=10,
):
    """Tuned fp8e4 DoubleRow variant.

    x is shipped as a hi fp8 plane over all of K plus a lo (residual) fp8
    plane over the first kpl/(in_f/256) fraction of K; both accumulate into
    the same PSUM group, so accuracy ~= e4m3 on the uncorrected tail only.
    Weights are host-binarized to exact +-1 fp8; alpha/bias applied in the
    DVE epilogue.

    Scheduling follows build_nc_hb2: chunk-resident x tiles (per-kp tags,
    double buffered), ns-outer / k-inner matmul loop for pipelined PSUM
    drains, output DMAs on the scalar queue.
    """
    f32 = mybir.dt.float32
    fp8 = mybir.dt.float8e4
    DR = mybir.MatmulPerfMode.DoubleRow

    assert in_f % (2 * P) == 0 and n_rows % n_chunk == 0 and n_chunk % P == 0
    OCH = max(1, o_shard // 512)
    o_mm = o_shard // OCH
    assert o_mm <= 512 and o_mm * OCH == o_shard
    KP = in_f // (2 * P)
    KPL = kpl
    assert 0 <= KPL <= KP
    in_f_lo = KPL * 2 * P
    NCH = n_rows // n_chunk
    NS = n_chunk // P
    psum_bufs = 2 if NS * OCH * 2 <= 8 else 1
    assert NS * OCH * psum_bufs <= 8

    nc = bacc.Bacc("TRN2", target_bir_lowering=False)

    xT = nc.declare_dram_parameter("xT", [in_f, n_rows], fp8, isOutput=False)
    Wb = nc.declare_dram_parameter("Wb", [in_f, o_shard], fp8, isOutput=False)
    a_rep = nc.declare_dram_parameter("a_rep", [P, o_shard], f32, isOutput=False)
    b_rep = nc.declare_dram_parameter("b_rep", [P, o_shard], f32, isOutput=False)
    if KPL:
        xL = nc.declare_dram_parameter(
            "xL", [in_f_lo, n_rows], fp8, isOutput=False
        )
        xL_t = xL[:].rearrange("(kp two p) n -> kp p two n", two=2, p=P)
    out = nc.declare_dram_parameter("out", [n_rows, o_shard], f32, isOutput=True)

    xT_t = xT[:].rearrange("(kp two p) n -> kp p two n", two=2, p=P)
    Wb_t = Wb[:].rearrange("(kp two p) o -> p kp two o", two=2, p=P)

    with tile.TileContext(nc) as tc:
        with (
            tc.tile_pool(name="consts", bufs=1) as consts,
            tc.tile_pool(name="xp", bufs=2) as xp,
            tc.tile_pool(name="outp", bufs=4) as outp,
            tc.tile_pool(name="psum", bufs=psum_bufs, space="PSUM") as psump,
        ):
            W_mm = consts.tile([P, KP, 2, o_shard], fp8)
            for kp in range(KP):
                w_eng = nc.scalar if kp % 2 == 0 else nc.gpsimd
                w_eng.dma_start(out=W_mm[:, kp], in_=Wb_t[:, kp])

            # alpha/bias are first needed at the first epilogue (~8us in);
            # load them after the weights so they don't delay chunk 0
            a_sb = consts.tile([P, o_shard], f32)
            nc.gpsimd.dma_start(out=a_sb[:], in_=a_rep[:])
            b_sb = consts.tile([P, o_shard], f32)
            nc.gpsimd.dma_start(out=b_sb[:], in_=b_rep[:])

            n_mm = KP + KPL
            for nch in range(NCH):
                x_hi, x_lo = [], []
                for kp in range(KP):
                    x_t = xp.tile([P, 2, n_chunk], fp8, tag=f"xh{kp}")
                    nc.sync.dma_start(
                        out=x_t[:],
                        in_=xT_t[kp, :, :, nch * n_chunk : (nch + 1) * n_chunk],
                    )
                    x_hi.append(x_t)
                for kp in range(KPL):
                    x_t = xp.tile([P, 2, n_chunk], fp8, tag=f"xl{kp}")
                    nc.sync.dma_start(
                        out=x_t[:],
                        in_=xL_t[kp, :, :, nch * n_chunk : (nch + 1) * n_chunk],
                    )
                    x_lo.append(x_t)
                for ns in range(NS):
                    psums = [
                        psump.tile(
                            [P, o_mm], f32,
                            tag=f"ps{ns}_{och}", name=f"ps{ns}_{och}",
                        )
                        for och in range(OCH)
                    ]
                    mm_i = 0
                    for tiles in (x_hi, x_lo):
                        for kp, x_t in enumerate(tiles):
                            for och in range(OCH):
                                nc.tensor.matmul(
                                    psums[och][:],
                                    x_t[:, :, ns * P : (ns + 1) * P],
                                    W_mm[:, kp, :, och * o_mm : (och + 1) * o_mm],
                                    start=(mm_i == 0),
                                    stop=(mm_i == n_mm - 1),
                                    perf_mode=DR,
                                )
                            mm_i += 1
                    o_sb = outp.tile([P, o_shard], f32, tag="o")
                    for och in range(OCH):
                        sl = slice(och * o_mm, (och + 1) * o_mm)
                        nc.vector.tensor_tensor(
                            o_sb[:, sl], psums[och][:], a_sb[:, sl],
                            mybir.AluOpType.mult,
                        )
                        nc.vector.tensor_tensor(
                            o_sb[:, sl], o_sb[:, sl], b_sb[:, sl],
                            mybir.AluOpType.add,
                        )
                    row0 = nch * n_chunk + ns * P
                    nc.scalar.dma_start(
                        out=out[row0 : row0 + P, :], in_=o_sb[:]
                    )
    nc.compile()
    return nc


def build_nc_dr3(
    n_rows=N_ROWS,
    in_f=IN_F,
    o_shard=O_SHARD,
    n_chunk=512,
    kpl=10,
    alpha_is_one=False,
    n_warm=12,
):
    """build_nc_dr2 plus startup/teardown tuning:

    - n_warm dummy DoubleRow matmuls on a memset scratch tile fill the
      ~5-10us DMA-queue-init window at kernel start, so the HAM clock gate
      reaches K=8/8 before the first real matmul (saves the half-clock
      ramp that otherwise covers the first ~10us of real work).
    - x tiles are shipped as half-chunks [128, 2, n_chunk/2]; the first
      ns-sweep of a chunk then depends on only half the chunk's bytes, so
      chunk 0 no longer starves a warm PE on a single DMA queue.
    - alpha_is_one=True drops the alpha multiply from the epilogue (the
      caller verifies alpha == 1 on the host): one DVE op per sweep
      instead of two, shortening the end-of-kernel drain chain.
    """
    f32 = mybir.dt.float32
    fp8 = mybir.dt.float8e4
    DR = mybir.MatmulPerfMode.DoubleRow

    assert in_f % (2 * P) == 0 and n_rows % n_chunk == 0 and n_chunk % (2 * P) == 0
    OCH = max(1, o_shard // 512)
    o_mm = o_shard // OCH
    assert o_mm <= 512 and o_mm * OCH == o_shard
    KP = in_f // (2 * P)
    KPL = kpl
    assert 0 <= KPL <= KP
    in_f_lo = KPL * 2 * P
    NCH = n_rows // n_chunk
    NS = n_chunk // P
    NH = n_chunk // 2  # half-chunk width
    psum_bufs = 2 if NS * OCH * 2 <= 8 else 1
    assert NS * OCH * psum_bufs <= 8

    nc = bacc.Bacc("TRN2", target_bir_lowering=False)

    xT = nc.declare_dram_parameter("xT", [in_f, n_rows], fp8, isOutput=False)
    Wb = nc.declare_dram_parameter("Wb", [in_f, o_shard], fp8, isOutput=False)
    if not alpha_is_one:
        a_rep = nc.declare_dram_parameter(
            "a_rep", [P, o_shard], f32, isOutput=False
        )
    b_rep = nc.declare_dram_parameter("b_rep", [P, o_shard], f32, isOutput=False)
    if KPL:
        xL = nc.declare_dram_parameter(
            "xL", [in_f_lo, n_rows], fp8, isOutput=False
        )
        xL_t = xL[:].rearrange("(kp two p) n -> kp p two n", two=2, p=P)
    out = nc.declare_dram_parameter("out", [n_rows, o_shard], f32, isOutput=True)

    xT_t = xT[:].rearrange("(kp two p) n -> kp p two n", two=2, p=P)
    Wb_t = Wb[:].rearrange("(kp two p) o -> p kp two o", two=2, p=P)

    with tile.TileContext(nc) as tc:
        with (
            tc.tile_pool(name="consts", bufs=1) as consts,
            tc.tile_pool(name="xp", bufs=2) as xp,
            tc.tile_pool(name="outp", bufs=4) as outp,
            tc.tile_pool(name="psum", bufs=psum_bufs, space="PSUM") as psump,
        ):
            # PE warm-up: dummy DR matmuls on zeroed scratch while the DMA
            # queues initialize and chunk 0 streams in.
            if n_warm:
                warm = consts.tile([P, 2, 512], fp8)
                nc.vector.memset(warm[:], 0.0)
                wps = psump.tile([P, o_mm], f32, tag="ps0_0", name="warm")
                for _ in range(n_warm):
                    nc.tensor.matmul(
                        wps[:], warm[:, :, :P], warm[:, :, :o_mm],
                        start=True, stop=True, perf_mode=DR,
                    )

            W_mm = consts.tile([P, KP, 2, o_shard], fp8)
            for kp in range(KP):
                w_eng = nc.scalar if kp % 2 == 0 else nc.gpsimd
                w_eng.dma_start(out=W_mm[:, kp], in_=Wb_t[:, kp])

            if not alpha_is_one:
                a_sb = consts.tile([P, o_shard], f32)
                nc.gpsimd.dma_start(out=a_sb[:], in_=a_rep[:])
            b_sb = consts.tile([P, o_shard], f32)
            nc.gpsimd.dma_start(out=b_sb[:], in_=b_rep[:])

            n_mm = KP + KPL
            for nch in range(NCH):
                x_hi = [[], []]
                x_lo = [[], []]
                for h in range(2):
                    col0 = nch * n_chunk + h * NH
                    for kp in range(KP):
                        x_t = xp.tile([P, 2, NH], fp8, tag=f"xh{h}_{kp}")
                        nc.sync.dma_start(
                            out=x_t[:], in_=xT_t[kp, :, :, col0 : col0 + NH]
                        )
                        x_hi[h].append(x_t)
                    for kp in range(KPL):
                        x_t = xp.tile([P, 2, NH], fp8, tag=f"xl{h}_{kp}")
                        nc.sync.dma_start(
                            out=x_t[:], in_=xL_t[kp, :, :, col0 : col0 + NH]
                        )
                        x_lo[h].append(x_t)
                for ns in range(NS):
                    h, hs = divmod(ns, NH // P)
                    psums = [
                        psump.tile(
                            [P, o_mm], f32,
                            tag=f"ps{ns}_{och}", name=f"ps{ns}_{och}",
                        )
                        for och in range(OCH)
                    ]
                    mm_i = 0
                    for tiles in (x_hi[h], x_lo[h]):
                        for kp, x_t in enumerate(tiles):
                            for och in range(OCH):
                                nc.tensor.matmul(
                                    psums[och][:],
                                    x_t[:, :, hs * P : (hs + 1) * P],
                                    W_mm[:, kp, :, och * o_mm : (och + 1) * o_mm],
                                    start=(mm_i == 0),
                                    stop=(mm_i == n_mm - 1),
                                    perf_mode=DR,
                                )
                            mm_i += 1
                    o_sb = outp.tile([P, o_shard], f32, tag="o")
                    for och in range(OCH):
                        sl = slice(och * o_mm, (och + 1) * o_mm)
                        if alpha_is_one:
                            nc.vector.tensor_tensor(
                                o_sb[:, sl], psums[och][:], b_sb[:, sl],
                                mybir.AluOpType.add,
                            )
                        else:
                            nc.vector.tensor_tensor(
                                o_sb[:, sl], psums[och][:], a_sb[:, sl],
                                mybir.AluOpType.mult,
                            )
                            nc.vector.tensor_tensor(
                                o_sb[:, sl], o_sb[:, sl], b_sb[:, sl],
                                mybir.AluOpType.add,
                            )
                    row0 = nch * n_chunk + ns * P
                    nc.scalar.dma_start(
                        out=out[row0 : row0 + P, :], in_=o_sb[:]
                    )
    nc.compile()
    return nc


def build_nc_dr4(
    n_rows=N_ROWS,
    in_f=IN_F,
    o_shard=O_SHARD,
    n_chunk=512,
    kpl=10,
    alpha_is_one=False,
    n_warm=12,
):
    """build_nc_dr2 + warm-up + fused epilogue + HOST-PRE-TILED x/W.

    The x stream's DMA queue is packet-rate limited (~330 packets/us); with
    the natural [in_f, n] layout each [128,2,n_chunk] tile costs 256
    packets of 512B (the DoubleRow pair dim breaks contiguity). Here the
    host ships x already tiled as [KP][NCH][128][2*n_chunk] so every
    partition line is one contiguous 1KB packet: 128 packets/tile, halving
    the queue's packet load (~160us floor vs ~343us), which removes all
    x-stream starvation. Same for the (small) weight tensor.

    Also: n_warm dummy DR matmuls on zeroed scratch warm the HAM clock
    gate during queue init; chunk 0's lo tiles ride the gpsimd queue so
    the first sweep isn't single-queue bound; alpha_is_one drops the
    epilogue multiply (caller checks alpha==1 on host).
    """
    f32 = mybir.dt.float32
    fp8 = mybir.dt.float8e4
    DR = mybir.MatmulPerfMode.DoubleRow

    assert in_f % (2 * P) == 0 and n_rows % n_chunk == 0 and n_chunk % P == 0
    OCH = max(1, o_shard // 512)
    o_mm = o_shard // OCH
    assert o_mm <= 512 and o_mm * OCH == o_shard
    KP = in_f // (2 * P)
    KPL = kpl
    assert 0 <= KPL <= KP
    NCH = n_rows // n_chunk
    NS = n_chunk // P
    psum_bufs = 2 if NS * OCH * 2 <= 8 else 1
    assert NS * OCH * psum_bufs <= 8

    nc = bacc.Bacc("TRN2", target_bir_lowering=False)

    xT = nc.declare_dram_parameter(
        "xT", [KP, NCH, P, 2, n_chunk], fp8, isOutput=False
    )
    Wb = nc.declare_dram_parameter(
        "Wb", [KP, P, 2, o_shard], fp8, isOutput=False
    )
    if not alpha_is_one:
        a_rep = nc.declare_dram_parameter(
            "a_rep", [P, o_shard], f32, isOutput=False
        )
    b_rep = nc.declare_dram_parameter("b_rep", [P, o_shard], f32, isOutput=False)
    if KPL:
        xL = nc.declare_dram_parameter(
            "xL", [KPL, NCH, P, 2, n_chunk], fp8, isOutput=False
        )
    out = nc.declare_dram_parameter("out", [n_rows, o_shard], f32, isOutput=True)

    with tile.TileContext(nc) as tc:
        with (
            tc.tile_pool(name="consts", bufs=1) as consts,
            tc.tile_pool(name="xp", bufs=2) as xp,
            tc.tile_pool(name="outp", bufs=4) as outp,
            tc.tile_pool(name="psum", bufs=psum_bufs, space="PSUM") as psump,
        ):
            if n_warm:
                warm = consts.tile([P, 2, 512], fp8)
                nc.vector.memset(warm[:], 0.0)
                wps = psump.tile([P, o_mm], f32, tag="ps0_0", name="warm")
                for _ in range(n_warm):
                    nc.tensor.matmul(
                        wps[:], warm[:, :, :P], warm[:, :, :o_mm],
                        start=True, stop=True, perf_mode=DR,
                    )

            W_mm = consts.tile([P, KP, 2, o_shard], fp8)
            for kp in range(KP):
                w_eng = nc.scalar if kp % 2 == 0 else nc.gpsimd
                w_eng.dma_start(out=W_mm[:, kp], in_=Wb[kp])

            # alpha/bias DMAs are issued after chunk 0's lo tiles below:
            # they are first needed at the first epilogue (~19us) while
            # chunk-0 lo feeds the PE from ~16us on the same gpsimd queue
            if not alpha_is_one:
                a_sb = consts.tile([P, o_shard], f32)
            b_sb = consts.tile([P, o_shard], f32)

            n_mm = KP + KPL
            for nch in range(NCH):
                x_hi, x_lo = [], []
                for kp in range(KP):
                    x_t = xp.tile([P, 2, n_chunk], fp8, tag=f"xh{kp}")
                    nc.sync.dma_start(out=x_t[:], in_=xT[kp, nch])
                    x_hi.append(x_t)
                for kp in range(KPL):
                    x_t = xp.tile([P, 2, n_chunk], fp8, tag=f"xl{kp}")
                    # lo tiles ride gpsimd (idle after the weight load):
                    # splits the x stream over two queues so neither the
                    # cold-start chunks nor steady state are queue-bound
                    nc.gpsimd.dma_start(out=x_t[:], in_=xL[kp, nch])
                    x_lo.append(x_t)
                if nch == 0:
                    if not alpha_is_one:
                        nc.gpsimd.dma_start(out=a_sb[:], in_=a_rep[:])
                    nc.gpsimd.dma_start(out=b_sb[:], in_=b_rep[:])
                for ns in range(NS):
                    psums = [
                        psump.tile(
                            [P, o_mm], f32,
                            tag=f"ps{ns}_{och}", name=f"ps{ns}_{och}",
                        )
                        for och in range(OCH)
                    ]
                    mm_i = 0
                    for tiles in (x_hi, x_lo):
                        for kp, x_t in enumerate(tiles):
                            for och in range(OCH):
                                nc.tensor.matmul(
                                    psums[och][:],
                                    x_t[:, :, ns * P : (ns + 1) * P],
                                    W_mm[:, kp, :, och * o_mm : (och + 1) * o_mm],
                                    start=(mm_i == 0),
                                    stop=(mm_i == n_mm - 1),
                                    perf_mode=DR,
                                )
                            mm_i += 1
                    o_sb = outp.tile([P, o_shard], f32, tag="o")
                    # epilogue must be on DVE: it is the only engine that
                    # can read PSUM besides Activation (gpsimd cannot)
                    e_eng = nc.vector
                    for och in range(OCH):
                        sl = slice(och * o_mm, (och + 1) * o_mm)
                        if alpha_is_one:
                            e_eng.tensor_tensor(
                                o_sb[:, sl], psums[och][:], b_sb[:, sl],
                                mybir.AluOpType.add,
                            )
                        else:
                            e_eng.tensor_tensor(
                                o_sb[:, sl], psums[och][:], a_sb[:, sl],
                                mybir.AluOpType.mult,
                            )
                            e_eng.tensor_tensor(
                                o_sb[:, sl], o_sb[:, sl], b_sb[:, sl],
                                mybir.AluOpType.add,
                            )
                    row0 = nch * n_chunk + ns * P
                    nc.scalar.dma_start(
                        out=out[row0 : row0 + P, :], in_=o_sb[:]
                    )
    nc.compile()
    return nc


def make_in_maps_dr4(
    x, W, alpha, b, n_cores=N_CORES, grid=(1, 8), lo_frac=0.0,
    n_chunk=512, alpha_is_one=False,
):
    """Host-side sharding + pre-tiling into the DoubleRow DMA layout."""
    import ml_dtypes

    e4 = ml_dtypes.float8_e4m3
    xs, ws = grid
    assert xs * ws == n_cores
    n_shard = x.shape[0] // xs
    o_shard = W.shape[0] // ws
    in_f = x.shape[1]
    KP = in_f // (2 * P)
    KPL = int(round(KP * lo_frac))
    in_f_lo = KPL * 2 * P
    NCH = n_shard // n_chunk

    xT32 = np.ascontiguousarray(x.T)
    xT = xT32.astype(e4)

    def tile5(a):  # [in_f_part, n_shard] -> [KP', NCH, P, 2, n_chunk]
        kp = a.shape[0] // (2 * P)
        return np.ascontiguousarray(
            a.reshape(kp, 2, P, NCH, n_chunk).transpose(0, 3, 2, 1, 4)
        )

    x_parts = [
        tile5(xT[:, r * n_shard : (r + 1) * n_shard]) for r in range(xs)
    ]
    if KPL:
        xL32 = xT32[:in_f_lo] - xT[:in_f_lo].astype(np.float32)
        xLf = xL32.astype(e4)
        xl_parts = [
            tile5(xLf[:, r * n_shard : (r + 1) * n_shard]) for r in range(xs)
        ]
    sgn = np.where(W >= 0, np.float32(1.0), np.float32(-1.0)).astype(e4)
    w_parts = {}
    in_maps = []
    for c in range(n_cores):
        r, q = divmod(c, ws)
        if q not in w_parts:
            sl = slice(q * o_shard, (q + 1) * o_shard)
            wT = np.ascontiguousarray(sgn[sl].T)  # [in_f, o_shard]
            w_parts[q] = {
                "Wb": np.ascontiguousarray(
                    wT.reshape(KP, 2, P, o_shard).transpose(0, 2, 1, 3)
                ),
                "b_rep": np.ascontiguousarray(
                    np.broadcast_to(
                        b[sl].reshape(1, -1).astype(np.float32), (P, o_shard)
                    )
                ),
            }
            if not alpha_is_one:
                w_parts[q]["a_rep"] = np.ascontiguousarray(
                    np.broadcast_to(
                        alpha[sl].reshape(1, -1).astype(np.float32),
                        (P, o_shard),
                    )
                )
        m = {"xT": x_parts[r], **w_parts[q]}
        if KPL:
            m["xL"] = xl_parts[r]
        in_maps.append(m)
    return in_maps


def make_in_maps_dr(x, W, alpha, b, n_cores=N_CORES, grid=(1, 8), lo_frac=0.0):
    import ml_dtypes

    e4 = ml_dtypes.float8_e4m3
    xs, ws = grid
    assert xs * ws == n_cores
    n_shard = x.shape[0] // xs
    o_shard = W.shape[0] // ws
    xT32 = np.ascontiguousarray(x.T)
    xT = xT32.astype(e4)
    in_f = x.shape[1]
    KP = in_f // (2 * P)
    KPL = int(round(KP * lo_frac))
    in_f_lo = KPL * 2 * P
    x_parts = [
        np.ascontiguousarray(xT[:, r * n_shard : (r + 1) * n_shard])
        for r in range(xs)
    ]
    if KPL:
        xL32 = xT32[:in_f_lo] - xT[:in_f_lo].astype(np.float32)
        xLf = xL32.astype(e4)
        xl_parts = [
            np.ascontiguousarray(xLf[:, r * n_shard : (r + 1) * n_shard])
            for r in range(xs)
        ]
    sgn = np.where(W >= 0, np.float32(1.0), np.float32(-1.0)).astype(e4)
    w_parts = {}
    in_maps = []
    for c in range(n_cores):
        r, q = divmod(c, ws)
        if q not in w_parts:
            sl = slice(q * o_shard, (q + 1) * o_shard)
            w_parts[q] = {
                "Wb": np.ascontiguousarray(sgn[sl].T),
                "a_rep": np.ascontiguousarray(
                    np.broadcast_to(
                        alpha[sl].reshape(1, -1).astype(np.float32),
                        (P, o_shard),
                    )
                ),
                "b_rep": np.ascontiguousarray(
                    np.broadcast_to(
                        b[sl].reshape(1, -1).astype(np.float32), (P, o_shard)
                    )
                ),
            }
        m = {"xT": x_parts[r], **w_parts[q]}
        if KPL:
            m["xL"] = xl_parts[r]
        in_maps.append(m)
    return in_maps


def make_in_maps_hb(x, W, alpha, b, n_cores=N_CORES, grid=(1, 8)):
    """Shard full inputs into per-core input maps (host-side only).

    Weights are binarized here: Wb = bf16(sign(W)) * bf16(alpha), matching
    the reference's sign(W)*alpha then the matmul-input bf16 rounding.
    """
    import ml_dtypes

    bf16 = ml_dtypes.bfloat16
    xs, ws = grid
    assert xs * ws == n_cores
    n_shard = x.shape[0] // xs
    o_shard = W.shape[0] // ws
    xT = np.ascontiguousarray(x.T).astype(bf16)
    x_parts = [
        np.ascontiguousarray(xT[:, r * n_shard : (r + 1) * n_shard])
        for r in range(xs)
    ]
    # sign in f32 (exact), multiply by alpha in f32, round once to bf16
    sgn = np.where(W >= 0, np.float32(1.0), np.float32(-1.0))
    bw = (sgn * alpha).astype(bf16)  # [out, in]
    w_parts = {}
    in_maps = []
    for c in range(n_cores):
        r, q = divmod(c, ws)
        if q not in w_parts:
            sl = slice(q * o_shard, (q + 1) * o_shard)
            w_parts[q] = {
                "Wb": np.ascontiguousarray(bw[sl].T),
                "b_rep": np.ascontiguousarray(
                    np.broadcast_to(
                        b[sl].reshape(1, -1).astype(np.float32), (P, o_shard)
                    )
                ),
            }
        in_maps.append({"xT": x_parts[r], **w_parts[q]})
    return in_maps


_NC_CACHE = {}


def kernel(x, W, alpha, b, trace=False, variant=VARIANT):
    x = np.asarray(x, dtype=np.float32)
    W = np.asarray(W, dtype=np.float32)
    alpha = np.asarray(alpha, dtype=np.float32)
    b = np.asarray(b, dtype=np.float32)

    n_rows, in_f = x.shape
    out_f = W.shape[0]
    grid = (1, 8)
    xs, ws = grid
    n_shard = n_rows // xs
    o_shard = out_f // ws

    # drE: dr3 (warm-up + half-tiles + fused epilogue when alpha == 1).
    # drA/B/C: dr2 with 10/11/12 of 16 k-pairs corrected.
    # dr1/dr15/dr2: first-cut DoubleRow probes.
    KPL_OF = {
        "drF": 10, "drE": 10, "drA": 10, "drB": 11, "drC": 12,
        "dr1": 0, "dr15": 8, "dr2": 16,
    }
    lo_frac = KPL_OF[variant] / 16.0 if variant in KPL_OF else 0.0
    alpha_is_one = bool(np.all(alpha == 1.0))

    key = (n_rows, in_f, variant, alpha_is_one)
    if key not in _NC_CACHE:
        if variant == "drF":
            _NC_CACHE[key] = build_nc_dr4(
                n_rows=n_shard, in_f=in_f, o_shard=o_shard,
                kpl=KPL_OF[variant], alpha_is_one=alpha_is_one,
            )
        elif variant == "drE":
            _NC_CACHE[key] = build_nc_dr3(
                n_rows=n_shard, in_f=in_f, o_shard=o_shard,
                kpl=KPL_OF[variant], alpha_is_one=alpha_is_one,
            )
        elif variant in ("drA", "drB", "drC"):
            _NC_CACHE[key] = build_nc_dr2(
                n_rows=n_shard, in_f=in_f, o_shard=o_shard,
                kpl=KPL_OF[variant],
            )
        elif variant.startswith("dr"):
            _NC_CACHE[key] = build_nc_dr(
                n_rows=n_shard, in_f=in_f, o_shard=o_shard, lo_frac=lo_frac
            )
        elif variant == "hb2":
            _NC_CACHE[key] = build_nc_hb2(
                n_rows=n_shard, in_f=in_f, o_shard=o_shard
            )
        else:
            _NC_CACHE[key] = build_nc_hb(
                n_rows=n_shard, in_f=in_f, o_shard=o_shard
            )
    nc = _NC_CACHE[key]

    if variant == "drF":
        in_maps = make_in_maps_dr4(
            x, W, alpha, b, grid=grid, lo_frac=lo_frac,
            alpha_is_one=alpha_is_one,
        )
    elif variant.startswith("dr"):
        in_maps = make_in_maps_dr(x, W, alpha, b, grid=grid, lo_frac=lo_frac)
        if variant == "drE" and alpha_is_one:
            in_maps = [
                {k: v for k, v in m.items() if k != "a_rep"} for m in in_maps
            ]
    else:
        in_maps = make_in_maps_hb(x, W, alpha, b, grid=grid)
    try:
        res = run_bass_kernel_spmd(
            nc, in_maps, core_ids=list(range(N_CORES)), trace=trace
        )
    except Exception:
        # The trace path needs antenv.axon_hooks + artifact upload, which
        # some containers lack. If we didn't ask for tracing ourselves,
        # retry once with tracing force-disabled instead of failing.
        if trace:
            raise
        os.environ["BASS_NEVER_TRACE"] = "1"
        res = run_bass_kernel_spmd(
            nc, in_maps, core_ids=list(range(N_CORES)), trace=False
        )
    full = np.empty((n_rows, out_f), dtype=np.float32)
    for c in range(N_CORES):
        r, q = divmod(c, ws)
        full[
            r * n_shard : (r + 1) * n_shard, q * o_shard : (q + 1) * o_shard
        ] = np.asarray(res.results[c]["out"])
    if trace:
        return full, res
    return full


if __name__ == "__main__":
    nc = build_nc_hb(n_rows=512, in_f=512, o_shard=256, n_chunk=256)
    print("build ok [hb]")
